# revision 1
# baseline (speedup 1.0000x reference)
"""NTM scatter-memory kernel for 8 Trainium2 NeuronCores (Bass/Tile), v3.

Row-sharded [8192,4096] memory, fp16 SBUF-resident shard per core.

Key structure:
  - single activation table set (exp_and_others: exp/tanh/square); sigmoid
    as 0.5*tanh(0.5x)+0.5; rsqrt via Newton on DVE seeded from the previous
    step (host provides exact step-0 seeds). No flash-max (cosines in [-1,1]).
  - fp16 memory/intermediates: fine enough ULP that the tiny rank-1 write
    survives requantization (bf16 loses it), 2x/4x DVE modes, 1 cyc/row PE.
  - dot products as TT(mult) + tensor_scalar-accum (scalar_tensor_tensor has
    no DVE accel modes); norms via ScalarE Square+accum on the other engine.
  - 2 collectives/step: AllGather[1,1] of the write-softmax partial sum
    (hidden under the s2 = a - e*mem precompute); AllGather[1,4100] fp16 with
    the unnormalized partial read + local read sum (1/G folded into the tanh
    scale of the X update).
  - read matmuls for half the columns ride inside the update loop (PSUM
    budget allows 4 chunks); latency-critical small DMAs go on the idle
    GPSIMD queue so they don't sit behind the weight-streaming DMAs.
"""

import numpy as np

M_SLOTS = 8192
N_DIM = 4096
FVS = 64
PLEN = 64
CDIM = 256
NIN, NOUT = 512, 512
NSTEPS = 8
EPS = 1e-8

N_CORES = 8
M_LOC = M_SLOTS // N_CORES          # 1024 rows per core
RT = M_LOC // 128                   # 8 row-tiles per core
NCH = N_DIM // 512                  # 8 column chunks of 512

MEM_BF16 = True                     # kept for test.py compat (means fp16 now)

_CACHE = {}


def build_nc(steps=NSTEPS, mem_bf16=MEM_BF16):
    import concourse.bacc as bacc
    import concourse.mybir as mybir
    import concourse.tile as tile
    from concourse.bass_isa import ReduceOp

    F32 = mybir.dt.float32
    F16 = mybir.dt.float16
    AL = mybir.AluOpType
    ACT = mybir.ActivationFunctionType

    try:
        import concourse.tile_utils as tile_utils
        tile_utils.max_sbuf_usage = 208 * 1024
    except Exception:
        pass

    nc = bacc.Bacc("TRN2", target_bir_lowering=False, debug=False,
                   num_devices=N_CORES)

    d_mem = nc.dram_tensor("mem", [128, RT * N_DIM], F16, kind="ExternalInput")
    d_rsn0 = nc.dram_tensor("rsn0", [128, RT], F32, kind="ExternalInput")
    d_rsk0 = nc.dram_tensor("rsk0", [128, 1], F32, kind="ExternalInput")
    d_x0 = nc.dram_tensor("x0col", [FVS, 1], F32, kind="ExternalInput")
    d_prog = nc.dram_tensor("progpad", [128, NSTEPS], F32, kind="ExternalInput")
    d_wct = nc.dram_tensor("wct", [128, CDIM], F32, kind="ExternalInput")
    d_bch = nc.dram_tensor("bchalf", [128, 2], F32, kind="ExternalInput")
    d_wt = nc.dram_tensor("wt", [CDIM, 3 * N_DIM], F16, kind="ExternalInput")
    d_wtb = nc.dram_tensor("wtb", [1, 3 * N_DIM], F16, kind="ExternalInput")
    d_kr = nc.dram_tensor("krall", [NSTEPS, N_DIM], F16, kind="ExternalInput")
    d_oe = nc.dram_tensor("oesb", [FVS, NOUT], F32, kind="ExternalInput")
    d_ones = nc.dram_tensor("onesrow", [1, 128], F16, kind="ExternalInput")
    d_out = nc.dram_tensor("out", [1, NOUT], F32, kind="ExternalOutput")

    RG = [list(range(N_CORES))]
    PW = N_DIM + 4                  # fp16 payload: 4096 rp + 1 lsum + pad 3

    from contextlib import ExitStack

    with tile.TileContext(nc) as tc:
        with ExitStack() as stack:
            ep = stack.enter_context
            pmem = ep(tc.tile_pool(name="pmem", bufs=1))
            ps2 = ep(tc.tile_pool(name="ps2", bufs=1))
            pconst = ep(tc.tile_pool(name="pconst", bufs=1))
            pstate = ep(tc.tile_pool(name="pstate", bufs=2))
            pvb = ep(tc.tile_pool(name="pvb", bufs=3))
            pprod = ep(tc.tile_pool(name="pprod", bufs=1))
            pwt = ep(tc.tile_pool(name="pwt", bufs=4))
            pwb = ep(tc.tile_pool(name="pwb", bufs=1))
            psm = ep(tc.tile_pool(name="psm", bufs=4))
            prow = ep(tc.tile_pool(name="prow", bufs=2))
            pr8 = ep(tc.tile_pool(name="pr8", bufs=1))
            prp = ep(tc.tile_pool(name="prp", bufs=1))
            pkr = ep(tc.tile_pool(name="pkr", bufs=1))
            pkrr = ep(tc.tile_pool(name="pkrr", bufs=1))
            pps = ep(tc.tile_pool(name="pps", bufs=2, space="PSUM"))
            ppsb = ep(tc.tile_pool(name="ppsb", bufs=4, space="PSUM"))
            ppsc = ep(tc.tile_pool(name="ppsc", bufs=1, space="PSUM"))
            pdram = ep(tc.tile_pool(name="pdram", bufs=4, space="DRAM"))

            # ---- persistent state ----
            mem = pmem.tile([128, RT * N_DIM], F16, tag="mem")
            nc.sync.dma_start(mem[:], d_mem[:])
            s2t = ps2.tile([128, RT * N_DIM], F16, tag="s2t")

            rs_n = pstate.tile([128, RT], F32, tag="rs_n")
            nc.sync.dma_start(rs_n[:], d_rsn0[:])
            rs_k = pstate.tile([128, 1], F32, tag="rs_k")
            nc.sync.dma_start(rs_k[:], d_rsk0[:])
            x_col = pstate.tile([FVS, 1], F32, tag="xcol")
            nc.sync.dma_start(x_col[:], d_x0[:])

            prog = pconst.tile([128, NSTEPS], F32, tag="prog")
            nc.sync.dma_start(prog[:], d_prog[:])
            wct = pconst.tile([128, CDIM], F32, tag="wct")
            nc.sync.dma_start(wct[:], d_wct[:])
            bch = pconst.tile([128, 2], F32, tag="bch")
            nc.sync.dma_start(bch[:], d_bch[:])
            oesb = pconst.tile([FVS, NOUT], F32, tag="oesb")
            nc.sync.dma_start(oesb[:], d_oe[:])
            onesb = pconst.tile([1, 128], F16, tag="onesb")
            nc.sync.dma_start(onesb[:], d_ones[:])

            def msl(rt):
                return slice(rt * N_DIM, (rt + 1) * N_DIM)

            def newton_rsqrt(y, x, iters, tagp, out=None):
                """rsqrt(x) by Newton from seed y (fp32)."""
                shape = list(y.shape)
                for i in range(iters):
                    t1 = psm.tile(shape, F32, tag=tagp + "nt")
                    nc.vector.tensor_tensor(t1[:], y[:], y[:], AL.mult)
                    nc.vector.tensor_tensor(t1[:], t1[:], x[:], AL.mult)
                    nc.vector.tensor_scalar(t1[:], t1[:], -0.5, 1.5,
                                            AL.mult, AL.add)
                    if out is not None and i == iters - 1:
                        nc.vector.tensor_tensor(out[:], y[:], t1[:], AL.mult)
                        return out
                    y2 = psm.tile(shape, F32, tag=tagp + "ny")
                    nc.vector.tensor_tensor(y2[:], y[:], t1[:], AL.mult)
                    y = y2
                return y

            def bcast_kr(dst, row_t):
                for ch in range(NCH):
                    krrow = pkrr.tile([1, 512], F16, tag="krrow")
                    nc.gpsimd.dma_start(
                        krrow[:],
                        d_kr[row_t:row_t + 1, ch * 512:(ch + 1) * 512])
                    kr_ps = pps.tile([128, 512], F32, tag="bc_ps")
                    nc.tensor.matmul(kr_ps[:], onesb[:], krrow[:],
                                     start=True, stop=True)
                    nc.scalar.copy(dst[:, ch * 512:(ch + 1) * 512], kr_ps[:])

            # kr broadcast for step 0
            kr_b = pkr.tile([128, N_DIM], F16, tag="krb")
            bcast_kr(kr_b, 0)

            kprefetch = []
            for t in range(steps):
                # ---------- controller ----------
                cat = psm.tile([128, 1], F32, tag="cat")
                nc.vector.tensor_copy(cat[FVS:128, :], prog[FVS:128, t:t + 1])
                nc.vector.tensor_copy(cat[0:FVS, :], x_col[:])
                c_ps = ppsc.tile([128, 2], F32, tag="mini")
                nc.tensor.matmul(c_ps[:, 0:1], wct[:, 0:128], cat[:],
                                 start=True, stop=True)
                nc.tensor.matmul(c_ps[:, 1:2], wct[:, 128:256], cat[:],
                                 start=True, stop=True)
                c_th = psm.tile([128, 2], F32, tag="c_th")
                for h in range(2):
                    nc.scalar.activation(c_th[:, h:h + 1], c_ps[:, h:h + 1],
                                         ACT.Tanh, bias=bch[:, h:h + 1],
                                         scale=0.5)
                c_sb = psm.tile([128, 2], F16, tag="c_sb")
                nc.vector.tensor_scalar(c_sb[:], c_th[:], 0.5, 0.5,
                                        AL.mult, AL.add)

                # ---------- k / e / a (k first so zw can start) ----------
                c0b = c_sb[:, 0:1].broadcast_to([128, 128])
                c1b = c_sb[:, 1:2].broadcast_to([128, 128])
                kea = []
                for m in (0, 1, 2):
                    vb = pvb.tile([128, N_DIM], F16, tag="vb")
                    wbm = pwb.tile([1, N_DIM], F16, tag="wbm")
                    nc.gpsimd.dma_start(
                        wbm[:], d_wtb[0:1, m * N_DIM:(m + 1) * N_DIM])
                    for ch in range(NCH):
                        cbase = m * N_DIM + ch * 512
                        if m == 0 and ch < len(kprefetch):
                            w0, w1 = kprefetch[ch]
                        else:
                            w0 = pwt.tile([128, 512], F16, tag="wtc")
                            nc.sync.dma_start(w0[:],
                                              d_wt[0:128, cbase:cbase + 512])
                            w1 = pwt.tile([128, 512], F16, tag="wtc")
                            nc.sync.dma_start(w1[:],
                                              d_wt[128:256, cbase:cbase + 512])
                        bc_ps = pps.tile([128, 512], F32, tag="bc_ps")
                        nc.tensor.matmul(bc_ps[:], c0b, w0[:],
                                         start=True, stop=False)
                        nc.tensor.matmul(bc_ps[:], c1b, w1[:],
                                         start=False, stop=False)
                        nc.tensor.matmul(bc_ps[:], onesb[:],
                                         wbm[0:1, ch * 512:(ch + 1) * 512],
                                         start=False, stop=True)
                        # m==1 (erase gate): sigmoid via tanh(0.5x)
                        nc.scalar.activation(vb[:, ch * 512:(ch + 1) * 512],
                                             bc_ps[:], ACT.Tanh,
                                             scale=0.5 if m == 1 else 1.0)
                    kea.append(vb)
                k_b, e_b, a_b = kea
                nc.vector.tensor_scalar(e_b[:], e_b[:], 0.5, 0.5,
                                        AL.mult, AL.add)

                # ---------- kk2 on DVE (early: softmax chain must be ready
                # to schedule the moment zw completes) ----------
                prod = pprod.tile([128, N_DIM], F16, tag="prod")
                kk2 = psm.tile([128, 1], F32, tag="kk2")
                nc.vector.tensor_tensor(prod[:], k_b[:], k_b[:], AL.mult)
                nc.vector.tensor_scalar(prod[:], prod[:], 1.0, None,
                                        AL.mult, AL.add, accum_out=kk2[:])
                rs_k = newton_rsqrt(rs_k, kk2, 7, "k")

                # ---------- zw = mem @ k ----------
                zw = psm.tile([128, RT], F32, tag="zw")
                for rt in range(RT):
                    nc.vector.tensor_tensor(prod[:], mem[:, msl(rt)], k_b[:],
                                            AL.mult)
                    nc.vector.tensor_scalar(prod[:], prod[:], 1.0, None,
                                            AL.mult, AL.add,
                                            accum_out=zw[:, rt:rt + 1])

                # ---------- write logits + local softmax sum ----------
                li_w = psm.tile([128, RT], F32, tag="li_w")
                nc.vector.tensor_tensor(li_w[:], zw[:], rs_n[:], AL.mult)
                nc.vector.tensor_scalar(li_w[:], li_w[:], rs_k[:], None,
                                        AL.mult)
                ex_w = psm.tile([128, RT], F32, tag="ex_w")
                nc.scalar.activation(ex_w[:], li_w[:], ACT.Exp)
                d8 = psm.tile([128, RT], F32, tag="d8")
                rsum = psm.tile([128, 1], F32, tag="rsum")
                nc.vector.tensor_scalar(d8[:], ex_w[:], 1.0, None,
                                        AL.mult, AL.add, accum_out=rsum[:])
                lsum = psm.tile([128, 1], F32, tag="lsum")
                nc.gpsimd.partition_all_reduce(lsum[:], rsum[:], 128,
                                               ReduceOp.add)

                # ---------- AllGather local write sums (gpsimd queue) ----
                ag1_in = pdram.tile([1, 1], F32, tag="ag1_in")
                ag1_out = pdram.tile([N_CORES, 1], F32, tag="ag1_out")
                nc.gpsimd.dma_start(ag1_in[:], lsum[0:1, :])
                nc.gpsimd.collective_compute(
                    "AllGather", AL.bypass, replica_groups=RG,
                    ins=[ag1_in.opt()], outs=[ag1_out.opt()])

                # ---------- s2 = a - e*mem (overlaps the AllGather) ------
                for rt in range(RT):
                    nc.vector.tensor_tensor(s2t[:, msl(rt)], mem[:, msl(rt)],
                                            e_b[:], AL.mult)
                for rt in range(RT):
                    nc.vector.tensor_tensor(s2t[:, msl(rt)], a_b[:],
                                            s2t[:, msl(rt)], AL.subtract)

                # ---------- global write sum -> w ----------
                st8 = psm.tile([N_CORES, 1], F32, tag="st8")
                nc.gpsimd.dma_start(st8[:], ag1_out[:])
                g8 = psm.tile([N_CORES, 1], F32, tag="g8")
                nc.gpsimd.partition_all_reduce(g8[:], st8[:], N_CORES,
                                               ReduceOp.add)
                gi1 = prow.tile([1, 1], F32, tag="gi1")
                nc.vector.reciprocal(gi1[:], g8[0:1, :])
                gib = psm.tile([128, 1], F32, tag="gib")
                nc.gpsimd.partition_broadcast(gib[:], gi1[:])
                w8 = psm.tile([128, RT], F32, tag="w8")
                nc.vector.tensor_scalar(w8[:], ex_w[:], gib[:], None, AL.mult)

                # ---------- update + zr + norms + read weights, per tile;
                # read matmuls for cols 0:2048 ride along (PSUM budget) ----
                zr = psm.tile([128, RT], F32, tag="zr")
                nsq = psm.tile([128, RT], F32, tag="nsq")
                rs_n_new = pstate.tile([128, RT], F32, tag="rs_n")
                li_r = psm.tile([128, RT], F32, tag="li_r")
                ex_r = psm.tile([128, RT], F32, tag="ex_r")
                u_bf = psm.tile([128, RT], F16, tag="u_bf")
                rp_ps = [ppsb.tile([96, 512], F32, tag="rp_ps",
                                   name=f"rp{t}_{i}") for i in range(3)]
                for rt in range(RT):
                    nc.vector.tensor_scalar(prod[:], s2t[:, msl(rt)],
                                            w8[:, rt:rt + 1], None, AL.mult)
                    nc.vector.tensor_tensor(mem[:, msl(rt)], mem[:, msl(rt)],
                                            prod[:], AL.add)
                    nc.vector.tensor_tensor(prod[:], mem[:, msl(rt)], kr_b[:],
                                            AL.mult)
                    nc.vector.tensor_scalar(prod[:], prod[:], 1.0, None,
                                            AL.mult, AL.add,
                                            accum_out=zr[:, rt:rt + 1])
                    # norms on ScalarE (dummy-out reuses the dead s2t slice)
                    nc.scalar.activation(s2t[:, msl(rt)], mem[:, msl(rt)],
                                         ACT.Square,
                                         accum_out=nsq[:, rt:rt + 1])
                    # per-tile read weight (1 Newton iter: seed drift ~2%)
                    newton_rsqrt(rs_n[:, rt:rt + 1], nsq[:, rt:rt + 1], 1,
                                 "n", out=rs_n_new[:, rt:rt + 1])
                    nc.vector.tensor_tensor(li_r[:, rt:rt + 1],
                                            zr[:, rt:rt + 1],
                                            rs_n_new[:, rt:rt + 1], AL.mult)
                    nc.scalar.activation(ex_r[:, rt:rt + 1],
                                         li_r[:, rt:rt + 1], ACT.Exp)
                    nc.vector.tensor_copy(u_bf[:, rt:rt + 1],
                                          ex_r[:, rt:rt + 1])
                    for cc in range(NCH):
                        po = (cc % 3) * 32
                        nc.tensor.matmul(
                            rp_ps[cc // 3][po:po + 1, :],
                            u_bf[:, rt:rt + 1],
                            mem[:, rt * N_DIM + cc * 512:
                                rt * N_DIM + cc * 512 + 512],
                            start=(rt == 0), stop=(rt == RT - 1))
                rs_n = rs_n_new

                # ---------- local read sum ----------
                rsum_r = psm.tile([128, 1], F32, tag="rsum_r")
                nc.vector.tensor_scalar(d8[:], ex_r[:], 1.0, None,
                                        AL.mult, AL.add, accum_out=rsum_r[:])
                lsum_r = psm.tile([128, 1], F32, tag="lsum_r")
                nc.gpsimd.partition_all_reduce(lsum_r[:], rsum_r[:], 128,
                                               ReduceOp.add)
                lsum16 = prow.tile([1, 1], F16, tag="lsum16")
                nc.vector.tensor_copy(lsum16[:], lsum_r[0:1, :])

                # ---------- remaining read matmuls (cols 2048:4096) ------
                pay_sb = prp.tile([1, PW], F16, tag="pay_sb")
                for cc in range(NCH):
                    po = (cc % 3) * 32
                    eng = nc.scalar.copy if cc % 2 == 0 else \
                        nc.vector.tensor_copy
                    eng(pay_sb[0:1, cc * 512:(cc + 1) * 512],
                        rp_ps[cc // 3][po:po + 1, :])
                nc.vector.tensor_copy(pay_sb[0:1, N_DIM:N_DIM + 1],
                                      lsum16[:])
                nc.vector.memset(pay_sb[0:1, N_DIM + 1:PW], 0.0)

                pay2 = pdram.tile([1, PW], F16, tag="pay2")
                ag2_out = pdram.tile([N_CORES, PW], F16, tag="ag2_out")
                nc.gpsimd.dma_start(pay2[:], pay_sb[:])

                # ---------- prefetch next kr broadcast + k weights ----
                kpre = []
                if t + 1 < steps:
                    kr_b2 = pkr.tile([128, N_DIM], F16, tag="krb")
                    bcast_kr(kr_b2, t + 1)
                    kr_b = kr_b2
                    for ch in range(2):
                        w0p = pwt.tile([128, 512], F16, tag="wtc")
                        nc.sync.dma_start(w0p[:],
                                          d_wt[0:128, ch * 512:ch * 512 + 512])
                        w1p = pwt.tile([128, 512], F16, tag="wtc")
                        nc.sync.dma_start(w1p[:],
                                          d_wt[128:256, ch * 512:ch * 512 + 512])
                        kpre.append((w0p, w1p))

                # ---------- AllGather read payloads; combine + X update --
                nc.gpsimd.collective_compute(
                    "AllGather", AL.bypass, replica_groups=RG,
                    ins=[pay2.opt()], outs=[ag2_out.opt()])

                r8 = pr8.tile([FVS, N_CORES * FVS], F16, tag="r8")
                for c in range(N_CORES):
                    eng = nc.gpsimd if c % 2 == 0 else nc.scalar
                    eng.dma_start(
                        r8[:, c * FVS:(c + 1) * FVS],
                        ag2_out[c:c + 1, 0:N_DIM].rearrange(
                            "one (i j) -> (one i) j", i=FVS))
                st16 = psm.tile([N_CORES, 1], F16, tag="st16")
                nc.gpsimd.dma_start(st16[:], ag2_out[:, N_DIM:N_DIM + 1])
                st8r = psm.tile([N_CORES, 1], F32, tag="st8")
                nc.vector.tensor_copy(st8r[:], st16[:])
                gr8 = psm.tile([N_CORES, 1], F32, tag="g8")
                nc.gpsimd.partition_all_reduce(gr8[:], st8r[:], N_CORES,
                                               ReduceOp.add)
                gir1 = prow.tile([1, 1], F32, tag="gi1")
                nc.vector.reciprocal(gir1[:], gr8[0:1, :])
                girb = psm.tile([FVS, 1], F32, tag="girb")
                nc.gpsimd.partition_broadcast(girb[:], gir1[:])

                r8f = pr8.tile([FVS, N_CORES * FVS], F32, tag="r8f")
                nc.vector.tensor_copy(r8f[:], r8[:])
                x_ps = ppsc.tile([FVS, 1], F32, tag="mini")
                for c in range(N_CORES):
                    nc.tensor.matmul(x_ps[:], r8f[:, c * FVS:(c + 1) * FVS],
                                     x_col[:], start=(c == 0),
                                     stop=(c == N_CORES - 1))
                x_new = pstate.tile([FVS, 1], F32, tag="xcol")
                nc.scalar.activation(x_new[:], x_ps[:], ACT.Tanh,
                                     scale=girb[:])
                x_col = x_new
                kprefetch = kpre

            # ---------- output: Xf @ output_embedding ----------
            o_ps = ppsc.tile([1, NOUT], F32, tag="mini")
            nc.tensor.matmul(o_ps[:], x_col[:], oesb[:], start=True, stop=True)
            o_sb = prow.tile([1, NOUT], F32, tag="o_sb")
            nc.vector.tensor_copy(o_sb[:], o_ps[:])
            nc.sync.dma_start(d_out[:], o_sb[:])

    nc.compile()
    return nc


def host_prep(inputs, mem_bf16=MEM_BF16):
    f16 = np.float16
    f32 = np.float32

    x = np.asarray(inputs["x"], f32)
    program = np.asarray(inputs["program"], f32)
    memory0 = np.asarray(inputs["memory0"], f32)
    ie = np.asarray(inputs["input_embedding"], f32)
    oe = np.asarray(inputs["output_embedding"], f32)
    Wc = np.asarray(inputs["Wc"], f32)
    bc = np.asarray(inputs["bc"], f32)
    Wk = np.asarray(inputs["Wk"], f32)
    bk = np.asarray(inputs["bk"], f32)
    We = np.asarray(inputs["We"], f32)
    be = np.asarray(inputs["be"], f32)
    Wa = np.asarray(inputs["Wa"], f32)
    ba = np.asarray(inputs["ba"], f32)
    Wrk = np.asarray(inputs["Wrk"], f32)
    brk = np.asarray(inputs["brk"], f32)

    x0 = (x @ ie).astype(f32)                   # [1, FVS]
    x0col = x0.reshape(FVS, 1)

    progpad = np.zeros((128, NSTEPS), f32)
    progpad[FVS:128, :] = program[0].T          # rows 64:128 = prog_t

    wct = np.ascontiguousarray(Wc.T)            # [128, 256]
    bchalf = np.ascontiguousarray(0.5 * bc.reshape(2, 128).T)

    wt = np.concatenate([Wk.T, We.T, Wa.T], axis=1).astype(f16)  # [256,12288]
    wtb = np.concatenate([bk, be, ba]).reshape(1, 3 * N_DIM).astype(f16)

    kr = np.tanh(program[0] @ Wrk.T + brk)      # [8, 4096]
    kr = kr / np.linalg.norm(kr, axis=1, keepdims=True)
    krall = kr.astype(f16)

    # exact step-0 rsqrt(||k||^2) seed (the t=0 controller chain is
    # host-computable because X_0 = x @ input_embedding)
    cat0 = np.concatenate([x0[0], program[0, 0]])
    c0 = 1.0 / (1.0 + np.exp(-(cat0 @ Wc.T + bc)))
    k0 = np.tanh(c0 @ Wk.T + bk)
    rsk0 = np.full((128, 1), 1.0 / np.linalg.norm(k0), f32)

    onesrow = np.ones((1, 128), f16)

    common = {
        "x0col": x0col, "progpad": progpad, "wct": wct, "bchalf": bchalf,
        "wt": wt, "wtb": wtb, "krall": krall, "rsk0": rsk0,
        "oesb": np.ascontiguousarray(oe), "onesrow": onesrow,
    }
    in_maps = []
    for r in range(N_CORES):
        shard = memory0[r * M_LOC:(r + 1) * M_LOC, :]
        n = np.sqrt((shard.astype(np.float64) ** 2).sum(1))
        rsn0 = np.ascontiguousarray(
            (1.0 / n).reshape(RT, 128).T.astype(f32))   # [p, rt]
        m = dict(common)
        m["mem"] = np.ascontiguousarray(
            shard.reshape(RT, 128, N_DIM).transpose(1, 0, 2)
            .reshape(128, RT * N_DIM).astype(f16))
        m["rsn0"] = rsn0
        in_maps.append(m)
    return in_maps


def kernel(**inputs):
    from concourse.bass_utils import run_bass_kernel_spmd
    key = ("nc", NSTEPS, MEM_BF16)
    if key not in _CACHE:
        _CACHE[key] = build_nc(NSTEPS, MEM_BF16)
    nc = _CACHE[key]
    in_maps = host_prep(inputs, MEM_BF16)
    res = run_bass_kernel_spmd(nc, in_maps, core_ids=list(range(N_CORES)))
    return np.asarray(res.results[0]["out"], np.float32)

